# revision 32
# baseline (speedup 1.0000x reference)
"""v7 hybrid: table-stationary one-hot matmul gather + ap_gather spill path.

Tensor path: the row table is packed as fp32 (entry = val + 4096*valid,
64 cols per row) and laid out in 196 aligned 128-row blocks (rows on
partitions). Each block serves up to 320 query slots: an int8
partition-broadcast DMA delivers the slots' rel-indices, DVE is_equal
builds a [row, slot] one-hot (fp32), and ONE matmul per block
(lhsT = table block [128 rows, 64 cols], rhs = one-hot [128, 320])
gathers all 320 rows at once into PSUM [64, 320]. Two blocks pack into
the upper/lower partition halves of one PSUM tile; one ACT copy evicts
both as int16. Host unpacks val/valid and transposes.

Spill path (gpsimd): queries beyond a block's 320 slots go through the
v4-style ap_gather over a striped fp16+valid-byte table (d=3).
"""

import numpy as np

P = 50
E = 2000
M = 64
F = 2_000_000
BASE = E + 2
PE = P * E
NCORES = 8
RPC = 25000             # rows per core
NSHARD = 64
RPS = 2 * PE // NSHARD  # 3125 rows per (core, group) shard
D = 3                   # gpsimd path: int32 words per partition per row
NIDX = 1024             # gpsimd path: slots per (core, group)
CHUNK = 512             # slots per ap_gather
NBLK = 196              # tensor path: 128-row blocks per core
NSL = 288               # query slots per block
EB = 4                  # pairs per output DMA (8 blocks)
CHL = 28                # table blocks per load chunk
PACK = 4096             # valid flag multiplier in packed table


def _build_row_table(facts_idx):
    fp = facts_idx[:, 0].astype(np.int64)
    fs = facts_idx[:, 1].astype(np.int64)
    fo = facts_idx[:, 2].astype(np.int64)
    h = (fp * BASE + fs) * BASE + fo
    ho = np.argsort(h, kind="stable")
    fp, fs, fo = fp[ho], fs[ho], fo[ho]

    def csr(keys, vals):
        order = np.argsort(keys, kind="stable")
        svals = vals[order].astype(np.int32)
        counts = np.bincount(keys, minlength=PE)
        off = np.zeros(PE + 1, np.int64)
        np.cumsum(counts, out=off[1:])
        return svals, off

    def windows(svals, off):
        starts = off[:-1]
        cnt = np.minimum(off[1:] - starts, M).astype(np.int32)
        gi = np.minimum(
            starts[:, None] + np.arange(M, dtype=np.int64)[None, :], F - 1
        )
        return svals[gi].astype(np.int32), cnt

    ps_vals, ps_off = csr(fp * E + fs, fo)
    po_vals, po_off = csr(fp * E + fo, fs)
    w_ps, c_ps = windows(ps_vals, ps_off)
    w_po, c_po = windows(po_vals, po_off)
    wins = np.concatenate([w_ps, w_po], axis=0)
    cnts = np.concatenate([c_ps, c_po], axis=0)
    valid = (np.arange(M, dtype=np.int32)[None, :] < cnts[:, None]).astype(
        np.uint8
    )
    return wins, valid


def _build_tables(facts_idx):
    wins, valid = _build_row_table(facts_idx)   # [2PE, 64] i32, [2PE, 64] u8
    packed = np.where(valid > 0, wins, -wins - 1).astype(np.float16)
    winsf = wins.astype(np.float16)

    aptabs, tPs = [], []
    for c in range(NCORES):
        V = winsf[c * RPC : (c + 1) * RPC].reshape(8, RPS, 16, 4)
        Vw = np.ascontiguousarray(V).view(np.int32)
        U = valid[c * RPC : (c + 1) * RPC].reshape(8, RPS, 16, 4)
        Uw = np.ascontiguousarray(U).view(np.int32)
        t = np.concatenate([Vw, Uw], axis=3)
        t = np.ascontiguousarray(t.transpose(0, 2, 1, 3))
        aptabs.append(t.reshape(128, RPS * D))

        Tc = np.zeros((NBLK * 128, 64), np.float16)
        Tc[:RPC] = packed[c * RPC : (c + 1) * RPC]
        tPs.append(np.ascontiguousarray(
            Tc.reshape(NBLK, 128, 64).transpose(1, 0, 2)
        ).reshape(128, NBLK * 64))
    return aptabs, tPs


def _route_queries(preds, bound_args, direction):
    rows = (preds.astype(np.int64) * E + bound_args.astype(np.int64)
            + np.where(direction == 0, 0, PE))
    cores = rows // RPC
    lrows = (rows % RPC).astype(np.int32)
    out = []
    for c in range(NCORES):
        sel = np.nonzero(cores == c)[0]
        lr = lrows[sel]
        order = np.argsort(lr, kind="stable")
        lr = lr[order]
        gq = sel[order].astype(np.int32)

        relf = np.zeros((NBLK, NSL), np.int8)
        wmap = np.full((NBLK, NSL), -1, np.int32)
        gfill = np.zeros(8, np.int32)
        idx_arr = np.zeros((8, NIDX), np.int16)
        qmap = np.full((8, NIDX), -1, np.int32)
        blk = (lr >> 7).astype(np.int32)
        starts = np.searchsorted(blk, np.arange(NBLK + 1))
        for b in range(NBLK):
            seg = slice(starts[b], starts[b + 1])
            rs = lr[seg]
            qs = gq[seg]
            k = min(NSL, rs.shape[0])
            relf[b, :k] = (rs[:k] & 127).astype(np.int8)
            wmap[b, :k] = qs[:k]
            for i in range(k, rs.shape[0]):
                r = int(rs[i])
                g = r // RPS
                f = gfill[g]
                if f >= NIDX:
                    raise RuntimeError(
                        f"gpsimd shard overflow core {c} group {g}"
                    )
                idx_arr[g, f] = r % RPS
                qmap[g, f] = int(qs[i])
                gfill[g] += 1
        out.append({
            "relf": relf.reshape(1, NBLK * NSL),
            "wmap": wmap,
            "idx": idx_arr,
            "qmap": qmap,
        })
    return out


def _wrap_idx(idx_core):
    out = np.empty((128, NIDX // 16), np.int16)
    for g in range(8):
        out[16 * g : 16 * g + 16, :] = idx_core[g].reshape(NIDX // 16, 16).T
    return out


def _build_nc():
    import concourse.bacc as bacc
    import concourse.mybir as mybir
    import concourse.tile as tile

    nc = bacc.Bacc("TRN2", target_bir_lowering=False, debug=False,
                   num_devices=1)
    dt = mybir.dt
    Alu = mybir.AluOpType
    NCH = NBLK // CHL   # 7 table load chunks
    NPAIR = NBLK // 2   # 98
    aptab_d = nc.dram_tensor("aptab", [128, RPS * D], dt.int32,
                             kind="ExternalInput")
    idx_d = nc.dram_tensor("idx", [128, NIDX // 16], dt.int16,
                           kind="ExternalInput")
    tP_d = nc.dram_tensor("tP", [128, NBLK * 64], dt.float16,
                          kind="ExternalInput")
    relf_d = nc.dram_tensor("relf", [1, NBLK * NSL], dt.int8,
                            kind="ExternalInput")
    rowid_d = nc.dram_tensor("rowid", [128, 1], dt.int8,
                             kind="ExternalInput")
    rowfull_d = nc.dram_tensor("rowfull", [128, 8 * NSL], dt.int8,
                               kind="ExternalInput")
    gout_d = nc.dram_tensor("gout", [128, NIDX * D], dt.int32,
                            kind="ExternalOutput")
    outT_d = nc.dram_tensor("outT", [128, NPAIR * NSL], dt.int16,
                            kind="ExternalOutput")

    with tile.TileContext(nc) as tc:
        with (
            tc.tile_pool(name="tp", bufs=1) as tp,
            tc.tile_pool(name="gp", bufs=2) as gp,
            tc.tile_pool(name="oh", bufs=4) as ohp,
            tc.tile_pool(name="ac", bufs=4) as acp,
            tc.tile_pool(name="rc", bufs=4) as rcp,
            tc.tile_pool(name="po", bufs=3, space="PSUM") as pop,
        ):
            rowid = tp.tile([128, 1], dt.int8)
            rowfull = tp.tile([128, 8 * NSL], dt.int8)
            idxt = tp.tile([128, NIDX // 16], dt.int16)
            aptab = tp.tile([128, RPS * D], dt.int32)
            tPc = [
                tp.tile([128, CHL * 64], dt.float16, name=f"tPc{i}")
                for i in range(NCH)
            ]
            nc.sync.dma_start(out=rowid[:], in_=rowid_d[:, :])
            nc.sync.dma_start(out=rowfull[:], in_=rowfull_d[:, :])
            nc.sync.dma_start(out=idxt[:], in_=idx_d[:, :])

            rcs = {}
            RELB = 8  # pairs per broadcast load
            NRB = (NPAIR + RELB - 1) // RELB

            def load_rel(rb):
                if 0 <= rb < NRB and rb not in rcs:
                    npr = min(RELB, NPAIR - rb * RELB)
                    t = rcp.tile([128, RELB * 2 * NSL], dt.int8, tag="r")
                    nc.sync.dma_start(
                        out=t[:, 0 : npr * 2 * NSL],
                        in_=relf_d[:, rb * RELB * 2 * NSL :
                                   (rb * RELB + npr) * 2 * NSL]
                        .to_broadcast([128, npr * 2 * NSL]),
                    )
                    rcs[rb] = t

            load_rel(0)
            load_rel(1)
            nc.gpsimd.dma_start(out=aptab[:], in_=aptab_d[:, :])
            nc.sync.dma_start(out=tPc[0][:], in_=tP_d[:, 0 : CHL * 64])
            for i in range(1, NCH):
                nc.sync.dma_start(
                    out=tPc[i][:],
                    in_=tP_d[:, i * CHL * 64 : (i + 1) * CHL * 64],
                )

            # ---- gpsimd stream ----
            for ch in range(NIDX // CHUNK):
                g = gp.tile([128, CHUNK * D], dt.int32, tag="g")
                nc.gpsimd.ap_gather(
                    out_ap=g[:].rearrange("p (i d) -> p i d", d=D),
                    in_ap=aptab[:].rearrange("p (i d) -> p i d", d=D),
                    idxs_ap=idxt[:, ch * (CHUNK // 16) : (ch + 1) * (CHUNK // 16)],
                    channels=128, num_elems=RPS, d=D, num_idxs=CHUNK,
                )
                nc.scalar.dma_start(
                    out=gout_d[:, ch * CHUNK * D : (ch + 1) * CHUNK * D],
                    in_=g[:],
                )

            # ---- tensor stream ----
            rowb2 = rowid[:, 0:1].to_broadcast([128, 4 * NSL])
            ohs = {}
            for pr in range(NPAIR):
                rb, roff = divmod(pr, RELB)
                if pr % 4 == 0:
                    npr4 = min(4, NPAIR - pr)
                    oh4 = ohp.tile([128, 8 * NSL], dt.float16, tag="oh")
                    nc.vector.tensor_tensor(
                        out=oh4[:, 0 : npr4 * 2 * NSL],
                        in0=rowfull[:, 0 : npr4 * 2 * NSL],
                        in1=rcs[rb][:, roff * 2 * NSL :
                                    (roff + npr4) * 2 * NSL],
                        op=Alu.is_equal,
                    )
                    ohs[pr] = oh4
                oh = ohs[pr - pr % 4][:, (pr % 4) * 2 * NSL :
                                      (pr % 4 + 1) * 2 * NSL]
                if roff == 0:
                    load_rel(rb + 2)
                if pr % 2 == 0:
                    po2 = pop.tile([128, 1024], dt.float32, tag="po")
                pbase = (pr % 2) * 512   # bank-aligned pair slot
                for half in range(2):
                    t = 2 * pr + half
                    tab = tPc[t // CHL]
                    col = (t % CHL) * 64
                    nc.tensor.matmul(
                        out=po2[64 * half : 64 * half + 64,
                                pbase : pbase + NSL],
                        lhsT=tab[:, col : col + 64],
                        rhs=oh[:, half * NSL : (half + 1) * NSL],
                        start=True, stop=True,
                    )
                ei = pr % EB
                if ei == 0:
                    eacc = acp.tile([128, EB * NSL], dt.int16, tag="e")
                if pr % 2 == 1:
                    nc.scalar.copy(
                        out=eacc[:, (ei - 1) * NSL : (ei + 1) * NSL]
                        .rearrange("p (b x) -> p b x", b=2),
                        in_=po2[:].rearrange("p (b x) -> p b x", b=2)[
                            :, :, 0:NSL
                        ],
                    )
                if ei == EB - 1:
                    ck = pr // EB
                    nc.sync.dma_start(
                        out=outT_d[:, ck * EB * NSL : (ck + 1) * EB * NSL],
                        in_=eacc[:],
                    )
            # NPAIR = 98 = 24*4 + 2: flush the remainder
            rem = NPAIR % EB
            if rem:
                nc.sync.dma_start(
                    out=outT_d[:, (NPAIR - rem) * NSL :],
                    in_=eacc[:, 0 : rem * NSL],
                )
    nc.compile()
    return nc


_NC_CACHE = None
LAST_RESULT = None


def kernel(facts_idx, preds, bound_args, direction):
    global _NC_CACHE, LAST_RESULT
    from concourse.bass_utils import run_bass_kernel_spmd

    facts_idx = np.asarray(facts_idx, dtype=np.int32)
    preds = np.asarray(preds, dtype=np.int32)
    bound_args = np.asarray(bound_args, dtype=np.int32)
    direction = np.asarray(direction, dtype=np.int32)
    n = preds.shape[0]

    aptabs, tPs = _build_tables(facts_idx)
    routes = _route_queries(preds, bound_args, direction)

    if _NC_CACHE is None:
        _NC_CACHE = _build_nc()
    nc = _NC_CACHE

    rowid = np.arange(128, dtype=np.int8).reshape(128, 1)
    rowfull = np.broadcast_to(rowid, (128, 8 * NSL)).copy()
    in_maps = []
    for c in range(NCORES):
        in_maps.append({
            "aptab": aptabs[c],
            "idx": _wrap_idx(routes[c]["idx"]),
            "tP": tPs[c],
            "relf": routes[c]["relf"],
            "rowid": rowid,
            "rowfull": rowfull,
        })
    res = run_bass_kernel_spmd(nc, in_maps, core_ids=list(range(NCORES)))
    LAST_RESULT = res

    NPAIR = NBLK // 2
    cand = np.empty((n, M), np.int32)
    valid = np.empty((n, M), np.uint8)
    for c in range(NCORES):
        r = routes[c]
        # gpsimd part
        ob = res.results[c]["gout"].reshape(8, 16, NIDX, D)
        cw = np.ascontiguousarray(
            ob[:, :, :, 0:2].transpose(0, 2, 1, 3)
        ).view(np.float16).reshape(8, NIDX, 64)
        vw = np.ascontiguousarray(
            ob[:, :, :, 2].transpose(0, 2, 1)
        ).view(np.uint8).reshape(8, NIDX, 64)
        ids = r["qmap"]
        m = ids >= 0
        cand[ids[m]] = cw[m].astype(np.int32)
        valid[ids[m]] = vw[m]
        # tensor part: outT [128, NPAIR, NSL] i16; partition c<64 = block
        # 2*pair col c, c>=64 = block 2*pair+1 col c-64
        ot = res.results[c]["outT"].reshape(128, NPAIR, NSL)
        w = np.ascontiguousarray(
            ot.reshape(2, 64, NPAIR, NSL).transpose(2, 0, 3, 1)
        ).reshape(NBLK, NSL, 64).astype(np.int32)
        vs = (w >= 0).astype(np.uint8)
        vals = np.where(w >= 0, w, -w - 1)
        wm = r["wmap"]
        m = wm >= 0
        cand[wm[m]] = vals[m]
        valid[wm[m]] = vs[m]
    return cand, valid.astype(bool)


# revision 33
# speedup vs baseline: 1.0099x; 1.0099x over previous
"""v7 hybrid: table-stationary one-hot matmul gather + ap_gather spill path.

Tensor path: the row table is packed as fp32 (entry = val + 4096*valid,
64 cols per row) and laid out in 196 aligned 128-row blocks (rows on
partitions). Each block serves up to 320 query slots: an int8
partition-broadcast DMA delivers the slots' rel-indices, DVE is_equal
builds a [row, slot] one-hot (fp32), and ONE matmul per block
(lhsT = table block [128 rows, 64 cols], rhs = one-hot [128, 320])
gathers all 320 rows at once into PSUM [64, 320]. Two blocks pack into
the upper/lower partition halves of one PSUM tile; one ACT copy evicts
both as int16. Host unpacks val/valid and transposes.

Spill path (gpsimd): queries beyond a block's 320 slots go through the
v4-style ap_gather over a striped fp16+valid-byte table (d=3).
"""

import numpy as np

P = 50
E = 2000
M = 64
F = 2_000_000
BASE = E + 2
PE = P * E
NCORES = 8
RPC = 25000             # rows per core
NSHARD = 64
RPS = 2 * PE // NSHARD  # 3125 rows per (core, group) shard
D = 3                   # gpsimd path: int32 words per partition per row
NIDX = 1024             # gpsimd path: slots per (core, group)
CHUNK = 512             # slots per ap_gather
NBLK = 196              # tensor path: 128-row blocks per core
NSL = 288               # query slots per block
EB = 4                  # pairs per output DMA (8 blocks)
CHL = 28                # table blocks per load chunk
PACK = 4096             # valid flag multiplier in packed table


def _build_row_table(facts_idx):
    fp = facts_idx[:, 0].astype(np.int64)
    fs = facts_idx[:, 1].astype(np.int64)
    fo = facts_idx[:, 2].astype(np.int64)
    h = (fp * BASE + fs) * BASE + fo
    ho = np.argsort(h, kind="stable")
    fp, fs, fo = fp[ho], fs[ho], fo[ho]

    def csr(keys, vals):
        order = np.argsort(keys, kind="stable")
        svals = vals[order].astype(np.int32)
        counts = np.bincount(keys, minlength=PE)
        off = np.zeros(PE + 1, np.int64)
        np.cumsum(counts, out=off[1:])
        return svals, off

    def windows(svals, off):
        starts = off[:-1]
        cnt = np.minimum(off[1:] - starts, M).astype(np.int32)
        gi = np.minimum(
            starts[:, None] + np.arange(M, dtype=np.int64)[None, :], F - 1
        )
        return svals[gi].astype(np.int32), cnt

    ps_vals, ps_off = csr(fp * E + fs, fo)
    po_vals, po_off = csr(fp * E + fo, fs)
    w_ps, c_ps = windows(ps_vals, ps_off)
    w_po, c_po = windows(po_vals, po_off)
    wins = np.concatenate([w_ps, w_po], axis=0)
    cnts = np.concatenate([c_ps, c_po], axis=0)
    valid = (np.arange(M, dtype=np.int32)[None, :] < cnts[:, None]).astype(
        np.uint8
    )
    return wins, valid


def _build_tables(facts_idx):
    wins, valid = _build_row_table(facts_idx)   # [2PE, 64] i32, [2PE, 64] u8
    packed = np.where(valid > 0, wins, -wins - 1).astype(np.float16)
    winsf = wins.astype(np.float16)

    aptabs, tPs = [], []
    for c in range(NCORES):
        V = winsf[c * RPC : (c + 1) * RPC].reshape(8, RPS, 16, 4)
        Vw = np.ascontiguousarray(V).view(np.int32)
        U = valid[c * RPC : (c + 1) * RPC].reshape(8, RPS, 16, 4)
        Uw = np.ascontiguousarray(U).view(np.int32)
        t = np.concatenate([Vw, Uw], axis=3)
        t = np.ascontiguousarray(t.transpose(0, 2, 1, 3))
        aptabs.append(t.reshape(128, RPS * D))

        Tc = np.zeros((NBLK * 128, 64), np.float16)
        Tc[:RPC] = packed[c * RPC : (c + 1) * RPC]
        tPs.append(np.ascontiguousarray(
            Tc.reshape(NBLK, 128, 64).transpose(1, 0, 2)
        ).reshape(128, NBLK * 64))
    return aptabs, tPs


def _route_queries(preds, bound_args, direction):
    rows = (preds.astype(np.int64) * E + bound_args.astype(np.int64)
            + np.where(direction == 0, 0, PE))
    cores = rows // RPC
    lrows = (rows % RPC).astype(np.int32)
    out = []
    for c in range(NCORES):
        sel = np.nonzero(cores == c)[0]
        lr = lrows[sel]
        order = np.argsort(lr, kind="stable")
        lr = lr[order]
        gq = sel[order].astype(np.int32)

        relf = np.zeros((NBLK, NSL), np.int8)
        wmap = np.full((NBLK, NSL), -1, np.int32)
        gfill = np.zeros(8, np.int32)
        idx_arr = np.zeros((8, NIDX), np.int16)
        qmap = np.full((8, NIDX), -1, np.int32)
        blk = (lr >> 7).astype(np.int32)
        starts = np.searchsorted(blk, np.arange(NBLK + 1))
        for b in range(NBLK):
            seg = slice(starts[b], starts[b + 1])
            rs = lr[seg]
            qs = gq[seg]
            k = min(NSL, rs.shape[0])
            relf[b, :k] = (rs[:k] & 127).astype(np.int8)
            wmap[b, :k] = qs[:k]
            for i in range(k, rs.shape[0]):
                r = int(rs[i])
                g = r // RPS
                f = gfill[g]
                if f >= NIDX:
                    raise RuntimeError(
                        f"gpsimd shard overflow core {c} group {g}"
                    )
                idx_arr[g, f] = r % RPS
                qmap[g, f] = int(qs[i])
                gfill[g] += 1
        out.append({
            "relf": relf.reshape(1, NBLK * NSL),
            "wmap": wmap,
            "idx": idx_arr,
            "qmap": qmap,
        })
    return out


def _wrap_idx(idx_core):
    out = np.empty((128, NIDX // 16), np.int16)
    for g in range(8):
        out[16 * g : 16 * g + 16, :] = idx_core[g].reshape(NIDX // 16, 16).T
    return out


def _build_nc():
    import concourse.bacc as bacc
    import concourse.mybir as mybir
    import concourse.tile as tile

    nc = bacc.Bacc("TRN2", target_bir_lowering=False, debug=False,
                   num_devices=1)
    dt = mybir.dt
    Alu = mybir.AluOpType
    NCH = NBLK // CHL   # 7 table load chunks
    NPAIR = NBLK // 2   # 98
    aptab_d = nc.dram_tensor("aptab", [128, RPS * D], dt.int32,
                             kind="ExternalInput")
    idx_d = nc.dram_tensor("idx", [128, NIDX // 16], dt.int16,
                           kind="ExternalInput")
    tP_d = nc.dram_tensor("tP", [128, NBLK * 64], dt.float16,
                          kind="ExternalInput")
    relf_d = nc.dram_tensor("relf", [1, NBLK * NSL], dt.int8,
                            kind="ExternalInput")
    rowid_d = nc.dram_tensor("rowid", [128, 1], dt.int8,
                             kind="ExternalInput")
    rowfull_d = nc.dram_tensor("rowfull", [128, 4 * NSL], dt.int8,
                               kind="ExternalInput")
    gout_d = nc.dram_tensor("gout", [128, NIDX * D], dt.int32,
                            kind="ExternalOutput")
    outT_d = nc.dram_tensor("outT", [128, NPAIR * NSL], dt.int16,
                            kind="ExternalOutput")

    with tile.TileContext(nc) as tc:
        with (
            tc.tile_pool(name="tp", bufs=1) as tp,
            tc.tile_pool(name="gp", bufs=2) as gp,
            tc.tile_pool(name="oh", bufs=8) as ohp,
            tc.tile_pool(name="ac", bufs=4) as acp,
            tc.tile_pool(name="rc", bufs=4) as rcp,
            tc.tile_pool(name="po", bufs=3, space="PSUM") as pop,
        ):
            rowid = tp.tile([128, 1], dt.int8)
            rowfull = tp.tile([128, 4 * NSL], dt.int8)
            idxt = tp.tile([128, NIDX // 16], dt.int16)
            aptab = tp.tile([128, RPS * D], dt.int32)
            tPc = [
                tp.tile([128, CHL * 64], dt.float16, name=f"tPc{i}")
                for i in range(NCH)
            ]
            nc.sync.dma_start(out=rowid[:], in_=rowid_d[:, :])
            nc.sync.dma_start(out=rowfull[:], in_=rowfull_d[:, :])
            nc.sync.dma_start(out=idxt[:], in_=idx_d[:, :])

            rcs = {}
            RELB = 8  # pairs per broadcast load
            NRB = (NPAIR + RELB - 1) // RELB

            def load_rel(rb):
                if 0 <= rb < NRB and rb not in rcs:
                    npr = min(RELB, NPAIR - rb * RELB)
                    t = rcp.tile([128, RELB * 2 * NSL], dt.int8, tag="r")
                    nc.sync.dma_start(
                        out=t[:, 0 : npr * 2 * NSL],
                        in_=relf_d[:, rb * RELB * 2 * NSL :
                                   (rb * RELB + npr) * 2 * NSL]
                        .to_broadcast([128, npr * 2 * NSL]),
                    )
                    rcs[rb] = t

            load_rel(0)
            load_rel(1)
            nc.gpsimd.dma_start(out=aptab[:], in_=aptab_d[:, :])
            nc.sync.dma_start(out=tPc[0][:], in_=tP_d[:, 0 : CHL * 64])
            for i in range(1, NCH):
                nc.sync.dma_start(
                    out=tPc[i][:],
                    in_=tP_d[:, i * CHL * 64 : (i + 1) * CHL * 64],
                )

            # ---- gpsimd stream ----
            for ch in range(NIDX // CHUNK):
                g = gp.tile([128, CHUNK * D], dt.int32, tag="g")
                nc.gpsimd.ap_gather(
                    out_ap=g[:].rearrange("p (i d) -> p i d", d=D),
                    in_ap=aptab[:].rearrange("p (i d) -> p i d", d=D),
                    idxs_ap=idxt[:, ch * (CHUNK // 16) : (ch + 1) * (CHUNK // 16)],
                    channels=128, num_elems=RPS, d=D, num_idxs=CHUNK,
                )
                nc.scalar.dma_start(
                    out=gout_d[:, ch * CHUNK * D : (ch + 1) * CHUNK * D],
                    in_=g[:],
                )

            # ---- tensor stream ----
            rowb2 = rowid[:, 0:1].to_broadcast([128, 4 * NSL])
            ohs = {}
            for pr in range(NPAIR):
                rb, roff = divmod(pr, RELB)
                if pr % 2 == 0:
                    npr2 = min(2, NPAIR - pr)
                    oh2 = ohp.tile([128, 4 * NSL], dt.float16, tag="oh")
                    nc.vector.tensor_tensor(
                        out=oh2[:, 0 : npr2 * 2 * NSL],
                        in0=rowfull[:, 0 : npr2 * 2 * NSL],
                        in1=rcs[rb][:, roff * 2 * NSL :
                                    (roff + npr2) * 2 * NSL],
                        op=Alu.is_equal,
                    )
                    ohs[pr] = oh2
                oh = ohs[pr - pr % 2][:, (pr % 2) * 2 * NSL :
                                      (pr % 2 + 1) * 2 * NSL]
                if roff == 0:
                    load_rel(rb + 2)
                if pr % 2 == 0:
                    po2 = pop.tile([128, 1024], dt.float32, tag="po")
                pbase = (pr % 2) * 512   # bank-aligned pair slot
                for half in range(2):
                    t = 2 * pr + half
                    tab = tPc[t // CHL]
                    col = (t % CHL) * 64
                    nc.tensor.matmul(
                        out=po2[64 * half : 64 * half + 64,
                                pbase : pbase + NSL],
                        lhsT=tab[:, col : col + 64],
                        rhs=oh[:, half * NSL : (half + 1) * NSL],
                        start=True, stop=True,
                    )
                ei = pr % EB
                if ei == 0:
                    eacc = acp.tile([128, EB * NSL], dt.int16, tag="e")
                if pr % 2 == 1:
                    nc.scalar.copy(
                        out=eacc[:, (ei - 1) * NSL : (ei + 1) * NSL]
                        .rearrange("p (b x) -> p b x", b=2),
                        in_=po2[:].rearrange("p (b x) -> p b x", b=2)[
                            :, :, 0:NSL
                        ],
                    )
                if ei == EB - 1:
                    ck = pr // EB
                    nc.sync.dma_start(
                        out=outT_d[:, ck * EB * NSL : (ck + 1) * EB * NSL],
                        in_=eacc[:],
                    )
            # NPAIR = 98 = 24*4 + 2: flush the remainder
            rem = NPAIR % EB
            if rem:
                nc.sync.dma_start(
                    out=outT_d[:, (NPAIR - rem) * NSL :],
                    in_=eacc[:, 0 : rem * NSL],
                )
    nc.compile()
    return nc


_NC_CACHE = None
LAST_RESULT = None


def kernel(facts_idx, preds, bound_args, direction):
    global _NC_CACHE, LAST_RESULT
    from concourse.bass_utils import run_bass_kernel_spmd

    facts_idx = np.asarray(facts_idx, dtype=np.int32)
    preds = np.asarray(preds, dtype=np.int32)
    bound_args = np.asarray(bound_args, dtype=np.int32)
    direction = np.asarray(direction, dtype=np.int32)
    n = preds.shape[0]

    aptabs, tPs = _build_tables(facts_idx)
    routes = _route_queries(preds, bound_args, direction)

    if _NC_CACHE is None:
        _NC_CACHE = _build_nc()
    nc = _NC_CACHE

    rowid = np.arange(128, dtype=np.int8).reshape(128, 1)
    rowfull = np.broadcast_to(rowid, (128, 4 * NSL)).copy()
    in_maps = []
    for c in range(NCORES):
        in_maps.append({
            "aptab": aptabs[c],
            "idx": _wrap_idx(routes[c]["idx"]),
            "tP": tPs[c],
            "relf": routes[c]["relf"],
            "rowid": rowid,
            "rowfull": rowfull,
        })
    res = run_bass_kernel_spmd(nc, in_maps, core_ids=list(range(NCORES)))
    LAST_RESULT = res

    NPAIR = NBLK // 2
    cand = np.empty((n, M), np.int32)
    valid = np.empty((n, M), np.uint8)
    for c in range(NCORES):
        r = routes[c]
        # gpsimd part
        ob = res.results[c]["gout"].reshape(8, 16, NIDX, D)
        cw = np.ascontiguousarray(
            ob[:, :, :, 0:2].transpose(0, 2, 1, 3)
        ).view(np.float16).reshape(8, NIDX, 64)
        vw = np.ascontiguousarray(
            ob[:, :, :, 2].transpose(0, 2, 1)
        ).view(np.uint8).reshape(8, NIDX, 64)
        ids = r["qmap"]
        m = ids >= 0
        cand[ids[m]] = cw[m].astype(np.int32)
        valid[ids[m]] = vw[m]
        # tensor part: outT [128, NPAIR, NSL] i16; partition c<64 = block
        # 2*pair col c, c>=64 = block 2*pair+1 col c-64
        ot = res.results[c]["outT"].reshape(128, NPAIR, NSL)
        w = np.ascontiguousarray(
            ot.reshape(2, 64, NPAIR, NSL).transpose(2, 0, 3, 1)
        ).reshape(NBLK, NSL, 64).astype(np.int32)
        vs = (w >= 0).astype(np.uint8)
        vals = np.where(w >= 0, w, -w - 1)
        wm = r["wmap"]
        m = wm >= 0
        cand[wm[m]] = vals[m]
        valid[wm[m]] = vs[m]
    return cand, valid.astype(bool)
